# revision 22
# baseline (speedup 1.0000x reference)
"""DampingGCN on 8 TRN2 NeuronCores.

Strategy (graph/data parallel per the sharding hint):
  - Nodes row-sharded: core c owns nodes [c*12500, (c+1)*12500), padded to
    12544 = 98 blocks of 128.
  - Edges partitioned by destination core, bucketed by (dst chunk of 4
    blocks, source quarter); bucket tile layout unified across the 8 cores
    (single SPMD program), pad positions gather row 0 and carry no S-matrix
    entry so they contribute nothing.
  - Per conv round: each core computes z = dinv * (h @ W) for its shard
    (TensorE), quantizes to fp8e4 and AllGathers the feature table Z in
    four quarter-tables (small first quarter so downstream gathers start
    early), then aggregates: dma_gather pulls Z[src] rows (256B fp8) into
    SBUF in 128-edge tiles; a one-hot selection matrix S (DVE is_equal of
    dst-slot vs iota) turns the per-destination segment-sum into TensorE
    matmuls accumulated in PSUM.  Consecutive tile pairs that target the
    same dst block are fused into a single fp8 DoubleRow matmul (2 k-tiles
    per instruction).  GCN normalization deg^-1/2 is folded: agg =
    dinv_d*(sum Z'[src] + Z'[d] + sqrt(deg)_d*bias) with Z' = dinv_s*z; the
    self-loop term Z'[d] is added with an identity matmul from the local z
    shard (no gather).
  - Descriptor generation for the gathers (the Q7/SWDGE bottleneck) is
    spread over all 4 SWDGE queues, which execute on independent Q7 cpu
    pairs concurrently (~3.4x effective).
  - The independent conv stacks (state c1/c2, phys p1/p2) are fused into
    256-wide rounds so each gathered row carries both; encoders run as
    512-column batched matmuls interleaved with round-0 z emission, and
    next-round z/AllGather emission is interleaved with aggregation chunks.
"""

import math
from contextlib import ExitStack

import numpy as np
import ml_dtypes

import concourse.bass as bass
import concourse.bacc as bacc
import concourse.tile as tile
from concourse import mybir
from concourse.bass_utils import run_bass_kernel_spmd

BF = ml_dtypes.bfloat16
F32 = mybir.dt.float32
BF16 = mybir.dt.bfloat16
FP8 = mybir.dt.float8e4
I16 = mybir.dt.int16
I32 = mybir.dt.int32

N = 100000
NCORES = 8
NC = N // NCORES          # 12500
NBLK = 98                 # dst blocks of 128 per core
PADN = NBLK * 128         # 12544
NT = NCORES * PADN        # 100352 global table rows
NSB = 4                   # source quarters (gather tables)
QBSTART = (0, 14, 42, 70, 98)      # quarter block boundaries
QROWS = (1792, 3584, 3584, 3584)   # local rows per quarter
CB = 4                    # dst blocks per superchunk (PSUM accumulators)
NCHUNK = math.ceil(NBLK / CB)
GCAP = 12                 # max 128-edge tiles per dma_gather
ZB = 7                    # z staging blocks per DMA
NQ = 4                    # SWDGE queues (concurrent Q7 cpu pairs)

AF = mybir.ActivationFunctionType
DR = mybir.MatmulPerfMode.DoubleRow

LAST_EXEC_NS = None
LAST_RESULTS = None


def _preprocess(edge_index):
    src = np.asarray(edge_index[0]).astype(np.int64)
    dst = np.asarray(edge_index[1]).astype(np.int64)
    deg = 1.0 + np.bincount(dst, minlength=N).astype(np.float64)
    dinv_full = (1.0 / np.sqrt(deg)).astype(np.float32)
    sqd_full = np.sqrt(deg).astype(np.float32)

    c_of = src // NC
    loc = src - c_of * NC
    qb = np.array(QBSTART) * 128
    qt_of = np.searchsorted(qb, loc, side="right") - 1   # source quarter
    qrows = np.array(QROWS)[qt_of]
    sblk_all = qt_of
    sidx_all = (c_of * qrows + loc - qb[qt_of]).astype(np.int64)
    core_of_dst = dst // NC

    # per-core edges keyed by (chunk, srcblk); block-major src-sorted inside
    NQS = NCHUNK * NSB
    cnts = np.zeros((NCORES, NQS), np.int64)
    percore = []
    for c in range(NCORES):
        m = core_of_dst == c
        dl = dst[m] - c * NC
        blk = dl // 128
        slot = dl % 128
        q = blk // CB
        key = q * NSB + sblk_all[m]
        order = np.lexsort((sidx_all[m], blk, key))
        percore.append((key[order], blk[order], slot[order],
                        sidx_all[m][order]))
        cnts[c] = np.bincount(key, minlength=NQS)

    T = np.max((cnts + 127) // 128, axis=0).astype(np.int64)  # [NQS] tiles
    toff = np.zeros(NQS + 1, np.int64)
    np.cumsum(T, out=toff[1:])
    TT = int(toff[-1])

    # place edges into the unified stream; record per-edge tile + block
    streams = []
    for c in range(NCORES):
        key_s, blk_s, slot_s, sidx_s = percore[c]
        starts = np.zeros(NQS + 1, np.int64)
        np.cumsum(cnts[c], out=starts[1:])
        rank = np.arange(len(key_s)) - starts[key_s]
        pos = toff[key_s] * 128 + rank
        streams.append((pos, blk_s, slot_s, sidx_s))

    # union (tile, block) pairs
    pair_seen = np.zeros((TT, NBLK), bool)
    for pos, blk_s, _, _ in streams:
        pair_seen[pos // 128, blk_s] = True

    # segments: cut each (q,s) tile run at GCAP
    segs = []  # (qs, tile0(global), ntiles)
    for qs in range(NQS):
        t = int(T[qs])
        for s0 in range(0, t, GCAP):
            segs.append((qs, int(toff[qs]) + s0, min(GCAP, t - s0)))
    NSEG = len(segs)

    # pair numbering ordered (seg, block, tile) so a block's pairs within a
    # segment are consecutive in both pair index and tile index — required
    # for the 2-ktile DoubleRow matmuls.
    pair_lut = np.full((TT, NBLK), -1, np.int64)
    seg_bgroups = []  # per seg: [(b, k0_local, npairs, pi0)]
    npair = 0
    for (qs, t0, nt) in segs:
        sub = pair_seen[t0:t0 + nt]  # [nt, NBLK]
        groups = []
        for b in np.nonzero(sub.any(axis=0))[0]:
            ks = np.nonzero(sub[:, b])[0]
            assert ks.max() - ks.min() + 1 == len(ks), "block tiles not contiguous"
            pi0 = npair
            for k in ks:
                pair_lut[t0 + k, b] = npair
                npair += 1
            groups.append((int(b), int(ks.min()), len(ks), pi0))
        seg_bgroups.append(groups)
    NPAIR = npair

    idx16 = np.zeros((NCORES, 128, TT * 8), np.int16)
    dstslot = np.full((NCORES, 128, NPAIR), -1.0, np.float32)
    for c in range(NCORES):
        pos, blk_s, slot_s, sidx_s = streams[c]
        # pad positions gather row 0 (valid, never consumed — no S pair);
        # keeps descriptor count == the decode-side ring reservation
        idx_arr = np.zeros(TT * 128, np.int16)
        idx_arr[pos] = sidx_s.astype(np.int16)
        pi = pair_lut[pos // 128, blk_s]
        assert (pi >= 0).all()
        plane = np.full(NPAIR * 128, -1.0, np.float32)
        plane[pi * 128 + pos % 128] = slot_s
        dstslot[c] = plane.reshape(NPAIR, 128).T
        idx16[c] = np.tile(idx_arr.reshape(-1, 16).T, (8, 1))

    # schedule
    schedule = []
    for q in range(NCHUNK):
        b0, b1 = q * CB, min((q + 1) * CB, NBLK)
        ct0, ct1 = int(toff[q * NSB]), int(toff[(q + 1) * NSB])
        gsegs = []
        for si_, (qs, t0, nt) in enumerate(segs):
            if qs // NSB != q:
                continue
            s = qs % NSB
            gsegs.append(dict(s=s, t0=t0, nt=nt, ci=si_,
                              bgroups=seg_bgroups[si_]))
        schedule.append(dict(blocks=list(range(b0, b1)), ct0=ct0, ct1=ct1,
                             segs=gsegs))

    dinv_p = np.ones((NCORES, PADN), np.float32)
    sqd_p = np.ones((NCORES, PADN), np.float32)
    for c in range(NCORES):
        sl = slice(c * NC, (c + 1) * NC)
        dinv_p[c, :NC] = dinv_full[sl]
        sqd_p[c, :NC] = sqd_full[sl]
    return dict(idx16=idx16, dstslot=dstslot, schedule=schedule, TT=TT,
                NSEG=NSEG, NPAIR=NPAIR, dinv_p=dinv_p, sqd_p=sqd_p)


def _build(schedule, TT, NSEG, NPAIR):
    nc = bacc.Bacc("TRN2", num_devices=NCORES, num_swdge_queues=NQ,
                   dynamic_dma_scratch_size=32768)
    P = {}

    def param(name, shape, dt):
        P[name] = nc.declare_dram_parameter(name, list(shape), dt, isOutput=False)
        return P[name]

    param("xat", (5, PADN), F32)
    param("dinvc", (128, NBLK), F32)
    param("sqdr", (1, PADN), BF16)
    param("idx16", (128, TT * 8), I16)
    param("dstslot", (128, NPAIR), BF16)
    param("iotab", (128, 2 * GCAP * 128), BF16)
    param("ident", (128, 128), BF16)
    param("ident8", (128, 128), FP8)
    param("se_W", (3, 128), F32)
    param("pe_W1", (2, 128), F32)
    for nm in ("pe_W2", "c1", "p1", "c2", "p2", "ccW0", "ccW1", "dw1"):
        param(nm, (128, 128), BF16)
    param("dw2", (128, 64), BF16)
    param("dw3", (64, 1), BF16)
    param("eb_se", (1, 128), BF16)
    param("eb_p1", (1, 128), BF16)
    param("eb_p2", (1, 128), BF16)
    param("cb1", (1, 256), BF16)
    param("cb2", (1, 256), BF16)
    param("cb3", (1, 128), BF16)
    param("db1", (1, 128), BF16)
    param("db2", (1, 64), BF16)
    param("db3", (1, 1), BF16)
    out_p = nc.declare_dram_parameter("out", [128, NBLK], F32, isOutput=True)

    with tile.TileContext(nc) as tc, ExitStack() as ctx:
        sing = ctx.enter_context(tc.tile_pool(name="sing", bufs=1))
        psp = ctx.enter_context(tc.tile_pool(name="psp", bufs=2, space="PSUM"))
        aggp = ctx.enter_context(tc.tile_pool(name="aggp", bufs=4, space="PSUM"))
        gp = ctx.enter_context(tc.tile_pool(name="gp", bufs=12))
        sp = ctx.enter_context(tc.tile_pool(name="sp", bufs=6))
        ip = ctx.enter_context(tc.tile_pool(name="ip", bufs=3))
        zp = ctx.enter_context(tc.tile_pool(name="zp", bufs=2))
        zs = ctx.enter_context(tc.tile_pool(name="zs", bufs=8))
        tp = ctx.enter_context(tc.tile_pool(name="tp", bufs=3))
        dramp = ctx.enter_context(tc.tile_pool(name="dramp", bufs=1, space="DRAM"))

        # ---------- constants (host-provided; keep gpsimd free) ----------
        sb = {}
        for nm, shape, dt in (
            ("dinvc", (128, NBLK), F32), ("sqdr", (1, PADN), BF16),
            ("dstslot", (128, NPAIR), BF16),
            ("ident", (128, 128), BF16), ("ident8", (128, 128), FP8),
            ("se_W", (3, 128), F32), ("pe_W1", (2, 128), F32),
            ("pe_W2", (128, 128), BF16),
            ("c1", (128, 128), BF16), ("p1", (128, 128), BF16),
            ("c2", (128, 128), BF16), ("p2", (128, 128), BF16),
            ("ccW0", (128, 128), BF16), ("ccW1", (128, 128), BF16),
            ("dw1", (128, 128), BF16), ("dw2", (128, 64), BF16),
            ("dw3", (64, 1), BF16),
            ("eb_se", (1, 128), BF16), ("eb_p1", (1, 128), BF16),
            ("eb_p2", (1, 128), BF16),
            ("cb1", (1, 256), BF16), ("cb2", (1, 256), BF16),
            ("cb3", (1, 128), BF16), ("db1", (1, 128), BF16),
            ("db2", (1, 64), BF16), ("db3", (1, 1), BF16),
        ):
            t = sing.tile(list(shape), dt, name=f"sb_{nm}")
            nc.sync.dma_start(t[:], P[nm].ap())
            sb[nm] = t
        iotab = sing.tile([128, 2 * GCAP, 128], BF16, name="iotab")
        nc.sync.dma_start(iotab[:], P["iotab"].ap())
        ident = sb["ident"]
        ident8 = sb["ident8"]
        ones_b = sing.tile([1, 128], BF16, name="ones_b")
        nc.vector.memset(ones_b[:], 1.0)

        h = sing.tile([128, NBLK, 256], BF16, name="h")
        cmb = h  # r2 output reuses h's low half (h is dead by then)
        outsb = sing.tile([128, NBLK], F32, name="outsb")

        zlocs = {}
        zgs = {}
        for r in range(3):
            zlocs[r] = [dramp.tile([QROWS[s] * 128 // 128, 256], FP8,
                                   name=f"zl{s}_{r}")
                        for s in range(4)]
            zgs[r] = [dramp.tile([NCORES * QROWS[s], 256], FP8,
                                 addr_space="Shared", name=f"zg{s}_{r}")
                      for s in range(4)]

        def quarter_of(b):
            for s in range(4):
                if b < QBSTART[s + 1]:
                    return s
            raise AssertionError(b)

        # scrub the G slots once so stale-SBUF NaNs can't leak through
        # zero-weighted pad rows
        for _ in range(12):
            gwarm = gp.tile([128, GCAP, 256], FP8, name="gwarm", tag="G")
            nc.vector.memset(gwarm[:], 0.0)

        gq = dict(load=[0] * NQ)

        def transpose_to(dst_sb, src_ap):
            pt = psp.tile([128, 128], BF16, name="pt", tag="pt")
            nc.tensor.transpose(pt[:], src_ap, ident[:])
            nc.scalar.activation(dst_sb, pt[:], AF.Copy)

        # ---------- encoders (batched: 4 blocks = 512 cols per matmul) ----------
        ones512 = sing.tile([1, 512], BF16, name="ones512")
        nc.vector.memset(ones512[:], 1.0)

        def enc_group(b0, nb):
            w = nb * 128
            gsl = slice(b0 * 128, b0 * 128 + w)
            xatg = tp.tile([3, 512], F32, name="xatg", tag="xat")
            nc.sync.dma_start(xatg[:, 0:w], P["xat"].ap()[0:3, gsl])
            attg = tp.tile([2, 512], F32, name="attg", tag="att")
            nc.sync.dma_start(attg[:, 0:w], P["xat"].ap()[3:5, gsl])
            ps = aggp.tile([128, 4, 128], F32, name="ps_enc", tag="agg")
            nc.tensor.matmul(ps[:, 0:nb, :], lhsT=sb["se_W"][:],
                             rhs=xatg[:, 0:w], start=True, stop=False)
            nc.tensor.matmul(ps[:, 0:nb, :], lhsT=sb["eb_se"][:],
                             rhs=ones512[:, 0:w], start=False, stop=True)
            nc.scalar.activation(h[:, b0:b0 + nb, 0:128], ps[:, 0:nb, :],
                                 AF.Sigmoid)
            # p0^T = sigmoid(pe_W1^T @ at + b1^T) computed directly
            pp = aggp.tile([128, 4, 128], F32, name="pp_enc", tag="agg")
            nc.tensor.matmul(pp[:, 0:nb, :], lhsT=sb["pe_W1"][:],
                             rhs=attg[:, 0:w], start=True, stop=False)
            nc.tensor.matmul(pp[:, 0:nb, :], lhsT=sb["eb_p1"][:],
                             rhs=ones512[:, 0:w], start=False, stop=True)
            p0t = tp.tile([128, 512], BF16, name="p0t", tag="tt")
            nc.scalar.activation(p0t[:, 0:w], pp[:, 0:nb, :], AF.Sigmoid)
            # note p0t is [f, n]; pe_W2^T @ p0 needs rhs = p0 [f(k), n] = p0t
            pq = aggp.tile([128, 4, 128], F32, name="pq_enc", tag="agg")
            nc.tensor.matmul(pq[:, 0:nb, :], lhsT=sb["pe_W2"][:],
                             rhs=p0t[:, 0:w], start=True, stop=False)
            nc.tensor.matmul(pq[:, 0:nb, :], lhsT=sb["eb_p2"][:],
                             rhs=ones512[:, 0:w], start=False, stop=True)
            nc.scalar.activation(h[:, b0:b0 + nb, 128:256], pq[:, 0:nb, :],
                                 AF.Sigmoid)

        # ---------- rounds ----------
        def emit_z_group(r, b0):
            """z for round r, blocks [b0, b0+7) (h already holds round-r
            input; transposed layout for r==0)."""
            fused = r < 2
            ncol = 256 if fused else 128
            zst = zp.tile([128, ZB, 256], FP8, name="zst", tag="zst")
            for bb in range(7):
                b = b0 + bb
                if r == 0:
                    sT = h[:, b, 0:128]
                    pT = h[:, b, 128:256]
                else:
                    sTt = tp.tile([128, 128], BF16, name="sT", tag="tt")
                    transpose_to(sTt[:], h[:, b, 0:128])
                    sT = sTt[:]
                    pTt = tp.tile([128, 128], BF16, name="pT", tag="tt")
                    transpose_to(pTt[:], h[:, b, 128:256])
                    pT = pTt[:]
                pz = psp.tile([128, ncol], F32, name="pz", tag="enc")
                if fused:
                    Wa = sb["c1"] if r == 0 else sb["c2"]
                    Wb = sb["p1"] if r == 0 else sb["p2"]
                    nc.tensor.matmul(pz[:, 0:128], lhsT=sT, rhs=Wa[:],
                                     start=True, stop=True)
                    nc.tensor.matmul(pz[:, 128:256], lhsT=pT, rhs=Wb[:],
                                     start=True, stop=True)
                else:
                    nc.tensor.matmul(pz[:], lhsT=sT, rhs=sb["ccW0"][:],
                                     start=True, stop=False)
                    nc.tensor.matmul(pz[:], lhsT=pT, rhs=sb["ccW1"][:],
                                     start=False, stop=True)
                nc.scalar.activation(zst[:, bb, 0:ncol], pz[:], AF.Copy,
                                     scale=sb["dinvc"][:, b:b + 1])
            bb = 0
            while bb < 7:
                s = quarter_of(b0 + bb)
                n = min(7 - bb, QBSTART[s + 1] - (b0 + bb))
                nc.sync.dma_start(
                    zlocs[r][s][:].rearrange("(t p) d -> p t d", p=128)
                    [:, b0 + bb - QBSTART[s]:b0 + bb - QBSTART[s] + n, :],
                    zst[:, bb:bb + n, :])
                bb += n

        AG_AT = {2: 0, 6: 1, 10: 2}  # z-group count -> quarter done (q3 lazy)
        ag_done = {0: set(), 1: set(), 2: set()}

        def emit_ag(r, s):
            ag_done[r].add(s)
            nc.gpsimd.collective_compute(
                "AllGather", mybir.AluOpType.bypass,
                replica_groups=[list(range(NCORES))],
                ins=[zlocs[r][s][:]], outs=[zgs[r][s][:]])

        def emit_head(b):
            cT = tp.tile([128, 128], BF16, name="cT", tag="tt")
            transpose_to(cT[:], cmb[:, b, 0:128])
            p1_ = psp.tile([128, 128], F32, name="p1h", tag="enc")
            nc.tensor.matmul(p1_[:], lhsT=sb["dw1"][:], rhs=cT[:],
                             start=True, stop=False)
            nc.tensor.matmul(p1_[:], lhsT=sb["db1"][:], rhs=ones_b[:],
                             start=False, stop=True)
            d1t = tp.tile([128, 128], BF16, name="d1t", tag="p0")
            nc.scalar.activation(d1t[:], p1_[:], AF.Sigmoid)
            p2_ = psp.tile([64, 128], F32, name="p2h", tag="enc")
            nc.tensor.matmul(p2_[:], lhsT=sb["dw2"][:], rhs=d1t[:],
                             start=True, stop=False)
            nc.tensor.matmul(p2_[:], lhsT=sb["db2"][:], rhs=ones_b[:],
                             start=False, stop=True)
            d2t = tp.tile([64, 128], BF16, name="d2t", tag="tt")
            nc.scalar.activation(d2t[:], p2_[:], AF.Sigmoid)
            p3_ = psp.tile([128, 1], F32, name="p3h", tag="enc")
            nc.tensor.matmul(p3_[:], lhsT=d2t[:], rhs=sb["dw3"][:],
                             start=True, stop=False)
            nc.tensor.matmul(p3_[:], lhsT=ones_b[:], rhs=sb["db3"][:],
                             start=False, stop=True)
            nc.scalar.activation(outsb[:, b:b + 1], p3_[:], AF.Sigmoid)

        def do_agg(r, on_chunk):
            fused = r < 2
            ncol = 256 if fused else 128
            bias = {0: "cb1", 1: "cb2", 2: "cb3"}[r]

            def load_islab(qi):
                ct0, ct1 = schedule[qi]["ct0"], schedule[qi]["ct1"]
                t = ip.tile([128, (ct1 - ct0) * 8], I16, name="islab",
                            tag="islab")
                nc.sync.dma_start(t[:], P["idx16"].ap()[:, ct0 * 8:ct1 * 8])
                return t
            nxt = load_islab(0)
            for qi, ch in enumerate(schedule):
                ps_of = {}
                # last (seg id, pair-chunk index) per block, for stop=
                last_of = {}
                for g in ch["segs"]:
                    for (b, k0, cnt, pi0) in g["bgroups"]:
                        last_of[b] = (id(g), b, (cnt - 1) // 2)
                banks = [aggp.tile([128, 512], F32, name="pagg", tag="agg")
                         for _ in range((len(ch["blocks"]) + 1) // 2)]
                for i, b in enumerate(ch["blocks"]):
                    pa = banks[i // 2][:, (i % 2) * ncol:(i % 2 + 1) * ncol]
                    ps_of[b] = pa
                    gsl = slice(b * 128, (b + 1) * 128)
                    nc.tensor.matmul(pa, lhsT=sb["sqdr"][:, gsl],
                                     rhs=sb[bias][:, 0:ncol],
                                     start=True, stop=False)
                    zself = zs.tile([128, 256], FP8, name="zself", tag="zself")
                    sq = quarter_of(b)
                    nc.sync.dma_start(
                        zself[:],
                        zlocs[r][sq][:].rearrange("(t p) d -> p t d", p=128)
                        [:, b - QBSTART[sq], :])
                    stop_now = b not in last_of
                    nc.tensor.matmul(pa, lhsT=ident8[:], rhs=zself[:, 0:ncol],
                                     start=False, stop=stop_now)
                ct0 = ch["ct0"]
                islab = nxt
                if qi + 1 < len(schedule):
                    nxt = load_islab(qi + 1)
                for g in ch["segs"]:
                    nt, t0 = g["nt"], g["t0"]
                    if g["s"] not in ag_done[r]:
                        emit_ag(r, g["s"])
                    # least-loaded queue (descriptor count proxy) keeps the
                    # four Q7 pairs evenly busy
                    q = min(range(NQ), key=lambda i: gq["load"][i])
                    gq["load"][q] += nt
                    G = gp.tile([128, GCAP, 256], FP8, name="G", tag="G")
                    nc.gpsimd.dma_gather(
                        G[:, 0:nt, :], zgs[r][g["s"]][:],
                        islab[:, (t0 - ct0) * 8:(t0 - ct0 + nt) * 8],
                        nt * 128, nt * 128, 256, single_packet=False,
                        queue_num=q)
                    if not g["bgroups"]:
                        continue
                    p0 = g["bgroups"][0][3]
                    np_ = sum(cnt for (_, _, cnt, _) in g["bgroups"])
                    assert np_ <= 2 * GCAP
                    S = sp.tile([128, 2 * GCAP, 128], FP8, name="S", tag="S")
                    nc.vector.tensor_tensor(
                        out=S[:, 0:np_, :],
                        in0=sb["dstslot"][:, p0:p0 + np_].to_broadcast(
                            [128, np_, 128]),
                        in1=iotab[:, 0:np_, :],
                        op=mybir.AluOpType.is_equal)
                    for (b, k0, cnt, pi0) in g["bgroups"]:
                        j0 = pi0 - p0
                        for j in range(0, cnt, 2):
                            stop = last_of[b] == (id(g), b, j // 2)
                            if j + 1 < cnt:
                                nc.tensor.matmul(
                                    ps_of[b],
                                    lhsT=S[:, j0 + j:j0 + j + 2, :],
                                    rhs=G[:, k0 + j:k0 + j + 2, 0:ncol],
                                    start=False, stop=stop, perf_mode=DR)
                            else:
                                nc.tensor.matmul(
                                    ps_of[b], lhsT=S[:, j0 + j, :],
                                    rhs=G[:, k0 + j, 0:ncol],
                                    start=False, stop=stop)
                for b in ch["blocks"]:
                    dst = h[:, b, 0:256] if fused else cmb[:, b, 0:128]
                    nc.scalar.activation(dst, ps_of[b], AF.Relu,
                                         scale=sb["dinvc"][:, b:b + 1])
                on_chunk(qi)

        zg0 = 0
        for b0 in range(0, NBLK, 4):
            nb = min(4, NBLK - b0)
            enc_group(b0, nb)
            while zg0 < 14 and 7 * zg0 + 7 <= b0 + nb:
                emit_z_group(0, 7 * zg0)
                zg0 += 1
                if zg0 == 2:
                    emit_ag(0, 0)

        for r in range(3):
            state = dict(zg=0, hd=0)

            def on_chunk(qi, r=r, state=state):
                ready = min(CB * (qi + 1), NBLK)
                if r < 2:
                    while state["zg"] < 14 and 7 * state["zg"] + 7 <= ready:
                        emit_z_group(r + 1, 7 * state["zg"])
                        state["zg"] += 1
                        if state["zg"] in AG_AT:
                            emit_ag(r + 1, AG_AT[state["zg"]])
                else:
                    while state["hd"] < NBLK and state["hd"] < ready:
                        emit_head(state["hd"])
                        state["hd"] += 1

            do_agg(r, on_chunk)

        nc.sync.dma_start(out_p.ap(), outsb[:])
    nc.compile()
    return nc


def kernel(**inputs):
    inputs = {k: np.asarray(v) for k, v in inputs.items()}
    pre = _preprocess(inputs["edge_index"])
    nc = _build(pre["schedule"], pre["TT"], pre["NSEG"], pre["NPAIR"])

    x = inputs["x"].astype(np.float32)
    alpha = inputs["alpha"].astype(np.float32)
    torque = inputs["torque"].astype(np.float32)

    def bf(a):
        return np.ascontiguousarray(np.asarray(a, np.float32).astype(BF))

    np8 = mybir.dt.np(FP8)
    iota_host = np.tile(np.arange(128, dtype=np.float32)[None, None, :],
                        (128, 2 * GCAP, 1)).reshape(128, 2 * GCAP * 128)
    shared = dict(
        se_W=np.ascontiguousarray(inputs["se_W"].astype(np.float32)),
        pe_W1=np.ascontiguousarray(inputs["pe_W1"].astype(np.float32)),
        pe_W2=bf(inputs["pe_W2"]), c1=bf(inputs["c1_W"]), p1=bf(inputs["p1_W"]),
        c2=bf(inputs["c2_W"]), p2=bf(inputs["p2_W"]),
        ccW0=bf(inputs["cc_W"][0:128]), ccW1=bf(inputs["cc_W"][128:256]),
        dw1=bf(inputs["dp_W1"]), dw2=bf(inputs["dp_W2"]), dw3=bf(inputs["dp_W3"]),
        eb_se=bf(inputs["se_b"][None, :]), eb_p1=bf(inputs["pe_b1"][None, :]),
        eb_p2=bf(inputs["pe_b2"][None, :]),
        cb1=bf(np.concatenate([inputs["c1_b"], inputs["p1_b"]])[None, :]),
        cb2=bf(np.concatenate([inputs["c2_b"], inputs["p2_b"]])[None, :]),
        cb3=bf(inputs["cc_b"][None, :]),
        db1=bf(inputs["dp_b1"][None, :]), db2=bf(inputs["dp_b2"][None, :]),
        db3=bf(inputs["dp_b3"][None, :]),
        iotab=bf(iota_host),
        ident=bf(np.eye(128, dtype=np.float32)),
        ident8=np.ascontiguousarray(np.eye(128, dtype=np.float32).astype(np8)),
    )

    in_maps = []
    for c in range(NCORES):
        sl = slice(c * NC, (c + 1) * NC)
        xat = np.zeros((5, PADN), np.float32)
        xat[0:3, :NC] = x[sl].T
        xat[3, :NC] = alpha[sl, 0]
        xat[4, :NC] = torque[sl, 0]
        dinvc = np.ascontiguousarray(pre["dinv_p"][c].reshape(NBLK, 128).T)
        in_maps.append(dict(
            shared,
            xat=xat, dinvc=dinvc,
            sqdr=np.ascontiguousarray(pre["sqd_p"][c][None, :].astype(BF)),
            idx16=np.ascontiguousarray(pre["idx16"][c]),
            dstslot=np.ascontiguousarray(pre["dstslot"][c].astype(BF)),
        ))

    import os
    trace = os.environ.get("KERNEL_PROFILE", "") == "1"
    res = run_bass_kernel_spmd(nc, in_maps, list(range(NCORES)), trace=trace)
    global LAST_EXEC_NS, LAST_RESULTS
    LAST_EXEC_NS = res.exec_time_ns
    LAST_RESULTS = res
    out = np.empty((N, 1), np.float32)
    for c in range(NCORES):
        o = res.results[c]["out"]  # [128, NBLK]
        out[c * NC:(c + 1) * NC, 0] = o.T.reshape(-1)[:NC]
    return out


# revision 24
# speedup vs baseline: 1.0060x; 1.0060x over previous
"""DampingGCN on 8 TRN2 NeuronCores.

Strategy (graph/data parallel per the sharding hint):
  - Nodes row-sharded: core c owns nodes [c*12500, (c+1)*12500), padded to
    12544 = 98 blocks of 128.
  - Edges partitioned by destination core, bucketed by (dst chunk of 4
    blocks, source quarter); bucket tile layout unified across the 8 cores
    (single SPMD program), pad positions gather row 0 and carry no S-matrix
    entry so they contribute nothing.
  - Per conv round: each core computes z = dinv * (h @ W) for its shard
    (TensorE), quantizes to fp8e4 and AllGathers the feature table Z in
    four quarter-tables (small first quarter so downstream gathers start
    early), then aggregates: dma_gather pulls Z[src] rows (256B fp8) into
    SBUF in 128-edge tiles; a one-hot selection matrix S (DVE is_equal of
    dst-slot vs iota) turns the per-destination segment-sum into TensorE
    matmuls accumulated in PSUM.  Consecutive tile pairs that target the
    same dst block are fused into a single fp8 DoubleRow matmul (2 k-tiles
    per instruction).  GCN normalization deg^-1/2 is folded: agg =
    dinv_d*(sum Z'[src] + Z'[d] + sqrt(deg)_d*bias) with Z' = dinv_s*z; the
    self-loop term Z'[d] is added with an identity matmul from the local z
    shard (no gather).
  - Descriptor generation for the gathers (the Q7/SWDGE bottleneck) is
    spread over all 4 SWDGE queues, which execute on independent Q7 cpu
    pairs concurrently (~3.4x effective).
  - The independent conv stacks (state c1/c2, phys p1/p2) are fused into
    256-wide rounds so each gathered row carries both; encoders run as
    512-column batched matmuls interleaved with round-0 z emission, and
    next-round z/AllGather emission is interleaved with aggregation chunks.
"""

import math
from contextlib import ExitStack

import numpy as np
import ml_dtypes

import concourse.bass as bass
import concourse.bacc as bacc
import concourse.tile as tile
from concourse import mybir
from concourse.bass_utils import run_bass_kernel_spmd

BF = ml_dtypes.bfloat16
F32 = mybir.dt.float32
BF16 = mybir.dt.bfloat16
FP8 = mybir.dt.float8e4
I16 = mybir.dt.int16
I32 = mybir.dt.int32

N = 100000
NCORES = 8
NC = N // NCORES          # 12500
NBLK = 98                 # dst blocks of 128 per core
PADN = NBLK * 128         # 12544
NT = NCORES * PADN        # 100352 global table rows
NSB = 4                   # source quarters (gather tables)
QBSTART = (0, 14, 42, 70, 98)      # quarter block boundaries
QROWS = (1792, 3584, 3584, 3584)   # local rows per quarter
CB = 4                    # dst blocks per superchunk (PSUM accumulators)
NCHUNK = math.ceil(NBLK / CB)
GCAP = 12                 # max 128-edge tiles per dma_gather
ZB = 7                    # z staging blocks per DMA
NQ = 4                    # SWDGE queues (concurrent Q7 cpu pairs)

AF = mybir.ActivationFunctionType
DR = mybir.MatmulPerfMode.DoubleRow

LAST_EXEC_NS = None
LAST_RESULTS = None


def _preprocess(edge_index):
    src = np.asarray(edge_index[0]).astype(np.int64)
    dst = np.asarray(edge_index[1]).astype(np.int64)
    deg = 1.0 + np.bincount(dst, minlength=N).astype(np.float64)
    dinv_full = (1.0 / np.sqrt(deg)).astype(np.float32)
    sqd_full = np.sqrt(deg).astype(np.float32)

    c_of = src // NC
    loc = src - c_of * NC
    qb = np.array(QBSTART) * 128
    qt_of = np.searchsorted(qb, loc, side="right") - 1   # source quarter
    qrows = np.array(QROWS)[qt_of]
    sblk_all = qt_of
    sidx_all = (c_of * qrows + loc - qb[qt_of]).astype(np.int64)
    core_of_dst = dst // NC

    # per-core edges keyed by (chunk, srcblk); block-major src-sorted inside
    NQS = NCHUNK * NSB
    cnts = np.zeros((NCORES, NQS), np.int64)
    percore = []
    for c in range(NCORES):
        m = core_of_dst == c
        dl = dst[m] - c * NC
        blk = dl // 128
        slot = dl % 128
        q = blk // CB
        key = q * NSB + sblk_all[m]
        order = np.lexsort((sidx_all[m], blk, key))
        percore.append((key[order], blk[order], slot[order],
                        sidx_all[m][order]))
        cnts[c] = np.bincount(key, minlength=NQS)

    T = np.max((cnts + 127) // 128, axis=0).astype(np.int64)  # [NQS] tiles
    toff = np.zeros(NQS + 1, np.int64)
    np.cumsum(T, out=toff[1:])
    TT = int(toff[-1])

    # place edges into the unified stream; record per-edge tile + block
    streams = []
    for c in range(NCORES):
        key_s, blk_s, slot_s, sidx_s = percore[c]
        starts = np.zeros(NQS + 1, np.int64)
        np.cumsum(cnts[c], out=starts[1:])
        rank = np.arange(len(key_s)) - starts[key_s]
        pos = toff[key_s] * 128 + rank
        streams.append((pos, blk_s, slot_s, sidx_s))

    # union (tile, block) pairs
    pair_seen = np.zeros((TT, NBLK), bool)
    for pos, blk_s, _, _ in streams:
        pair_seen[pos // 128, blk_s] = True

    # segments: cut each (q,s) tile run at GCAP
    segs = []  # (qs, tile0(global), ntiles)
    for qs in range(NQS):
        t = int(T[qs])
        for s0 in range(0, t, GCAP):
            segs.append((qs, int(toff[qs]) + s0, min(GCAP, t - s0)))
    NSEG = len(segs)

    # pair numbering ordered (seg, block, tile) so a block's pairs within a
    # segment are consecutive in both pair index and tile index — required
    # for the 2-ktile DoubleRow matmuls.
    pair_lut = np.full((TT, NBLK), -1, np.int64)
    seg_bgroups = []  # per seg: [(b, k0_local, npairs, pi0)]
    npair = 0
    for (qs, t0, nt) in segs:
        sub = pair_seen[t0:t0 + nt]  # [nt, NBLK]
        groups = []
        for b in np.nonzero(sub.any(axis=0))[0]:
            ks = np.nonzero(sub[:, b])[0]
            assert ks.max() - ks.min() + 1 == len(ks), "block tiles not contiguous"
            pi0 = npair
            for k in ks:
                pair_lut[t0 + k, b] = npair
                npair += 1
            groups.append((int(b), int(ks.min()), len(ks), pi0))
        seg_bgroups.append(groups)
    NPAIR = npair

    idx16 = np.zeros((NCORES, 128, TT * 8), np.int16)
    dstslot = np.full((NCORES, 128, NPAIR), -1.0, np.float32)
    for c in range(NCORES):
        pos, blk_s, slot_s, sidx_s = streams[c]
        # pad positions gather row 0 (valid, never consumed — no S pair);
        # keeps descriptor count == the decode-side ring reservation
        idx_arr = np.zeros(TT * 128, np.int16)
        idx_arr[pos] = sidx_s.astype(np.int16)
        pi = pair_lut[pos // 128, blk_s]
        assert (pi >= 0).all()
        plane = np.full(NPAIR * 128, -1.0, np.float32)
        plane[pi * 128 + pos % 128] = slot_s
        dstslot[c] = plane.reshape(NPAIR, 128).T
        idx16[c] = np.tile(idx_arr.reshape(-1, 16).T, (8, 1))

    # schedule
    schedule = []
    for q in range(NCHUNK):
        b0, b1 = q * CB, min((q + 1) * CB, NBLK)
        ct0, ct1 = int(toff[q * NSB]), int(toff[(q + 1) * NSB])
        gsegs = []
        for si_, (qs, t0, nt) in enumerate(segs):
            if qs // NSB != q:
                continue
            s = qs % NSB
            gsegs.append(dict(s=s, t0=t0, nt=nt, ci=si_,
                              bgroups=seg_bgroups[si_]))
        schedule.append(dict(blocks=list(range(b0, b1)), ct0=ct0, ct1=ct1,
                             segs=gsegs))

    dinv_p = np.ones((NCORES, PADN), np.float32)
    sqd_p = np.ones((NCORES, PADN), np.float32)
    for c in range(NCORES):
        sl = slice(c * NC, (c + 1) * NC)
        dinv_p[c, :NC] = dinv_full[sl]
        sqd_p[c, :NC] = sqd_full[sl]
    return dict(idx16=idx16, dstslot=dstslot, schedule=schedule, TT=TT,
                NSEG=NSEG, NPAIR=NPAIR, dinv_p=dinv_p, sqd_p=sqd_p)


def _build(schedule, TT, NSEG, NPAIR):
    nc = bacc.Bacc("TRN2", num_devices=NCORES, num_swdge_queues=NQ,
                   dynamic_dma_scratch_size=32768)
    P = {}

    def param(name, shape, dt):
        P[name] = nc.declare_dram_parameter(name, list(shape), dt, isOutput=False)
        return P[name]

    param("xat", (5, PADN), F32)
    param("dinvc", (128, NBLK), F32)
    param("sqdr", (1, PADN), BF16)
    param("idx16", (128, TT * 8), I16)
    param("dstslot", (128, NPAIR), BF16)
    param("iotab", (128, 2 * GCAP * 128), BF16)
    param("ident", (128, 128), BF16)
    param("ident8", (128, 128), FP8)
    param("se_W", (3, 128), F32)
    param("pe_W1", (2, 128), F32)
    for nm in ("pe_W2", "c1", "p1", "c2", "p2", "ccW0", "ccW1", "dw1"):
        param(nm, (128, 128), BF16)
    param("dw2", (128, 64), BF16)
    param("dw3", (64, 1), BF16)
    param("eb_se", (1, 128), BF16)
    param("eb_p1", (1, 128), BF16)
    param("eb_p2", (1, 128), BF16)
    param("cb1", (1, 256), BF16)
    param("cb2", (1, 256), BF16)
    param("cb3", (1, 128), BF16)
    param("db1", (1, 128), BF16)
    param("db2", (1, 64), BF16)
    param("db3", (1, 1), BF16)
    out_p = nc.declare_dram_parameter("out", [128, NBLK], F32, isOutput=True)

    with tile.TileContext(nc) as tc, ExitStack() as ctx:
        sing = ctx.enter_context(tc.tile_pool(name="sing", bufs=1))
        psp = ctx.enter_context(tc.tile_pool(name="psp", bufs=2, space="PSUM"))
        aggp = ctx.enter_context(tc.tile_pool(name="aggp", bufs=4, space="PSUM"))
        gp = ctx.enter_context(tc.tile_pool(name="gp", bufs=12))
        sp = ctx.enter_context(tc.tile_pool(name="sp", bufs=6))
        ip = ctx.enter_context(tc.tile_pool(name="ip", bufs=3))
        zp = ctx.enter_context(tc.tile_pool(name="zp", bufs=2))
        zs = ctx.enter_context(tc.tile_pool(name="zs", bufs=8))
        tp = ctx.enter_context(tc.tile_pool(name="tp", bufs=3))
        dramp = ctx.enter_context(tc.tile_pool(name="dramp", bufs=1, space="DRAM"))

        # ---------- constants (host-provided; keep gpsimd free) ----------
        sb = {}
        for nm, shape, dt in (
            ("dinvc", (128, NBLK), F32), ("sqdr", (1, PADN), BF16),
            ("dstslot", (128, NPAIR), BF16),
            ("ident", (128, 128), BF16), ("ident8", (128, 128), FP8),
            ("se_W", (3, 128), F32), ("pe_W1", (2, 128), F32),
            ("pe_W2", (128, 128), BF16),
            ("c1", (128, 128), BF16), ("p1", (128, 128), BF16),
            ("c2", (128, 128), BF16), ("p2", (128, 128), BF16),
            ("ccW0", (128, 128), BF16), ("ccW1", (128, 128), BF16),
            ("dw1", (128, 128), BF16), ("dw2", (128, 64), BF16),
            ("dw3", (64, 1), BF16),
            ("eb_se", (1, 128), BF16), ("eb_p1", (1, 128), BF16),
            ("eb_p2", (1, 128), BF16),
            ("cb1", (1, 256), BF16), ("cb2", (1, 256), BF16),
            ("cb3", (1, 128), BF16), ("db1", (1, 128), BF16),
            ("db2", (1, 64), BF16), ("db3", (1, 1), BF16),
        ):
            t = sing.tile(list(shape), dt, name=f"sb_{nm}")
            nc.sync.dma_start(t[:], P[nm].ap())
            sb[nm] = t
        iotab = sing.tile([128, 2 * GCAP, 128], BF16, name="iotab")
        nc.sync.dma_start(iotab[:], P["iotab"].ap())
        ident = sb["ident"]
        ident8 = sb["ident8"]
        ones_b = sing.tile([1, 128], BF16, name="ones_b")
        nc.vector.memset(ones_b[:], 1.0)

        h = sing.tile([128, NBLK, 256], BF16, name="h")
        cmb = h  # r2 output reuses h's low half (h is dead by then)
        outsb = sing.tile([128, NBLK], F32, name="outsb")

        zlocs = {}
        zgs = {}
        for r in range(3):
            zlocs[r] = [dramp.tile([QROWS[s] * 128 // 128, 256], FP8,
                                   name=f"zl{s}_{r}")
                        for s in range(4)]
            zgs[r] = [dramp.tile([NCORES * QROWS[s], 256], FP8,
                                 addr_space="Shared", name=f"zg{s}_{r}")
                      for s in range(4)]

        def quarter_of(b):
            for s in range(4):
                if b < QBSTART[s + 1]:
                    return s
            raise AssertionError(b)

        # scrub the G slots once so stale-SBUF NaNs can't leak through
        # zero-weighted pad rows
        for _ in range(12):
            gwarm = gp.tile([128, GCAP, 256], FP8, name="gwarm", tag="G")
            nc.vector.memset(gwarm[:], 0.0)

        gq = dict(load=[0] * NQ)

        def transpose_to(dst_sb, src_ap):
            pt = psp.tile([128, 128], BF16, name="pt", tag="pt")
            nc.tensor.transpose(pt[:], src_ap, ident[:])
            nc.scalar.activation(dst_sb, pt[:], AF.Copy)

        # ---------- encoders (batched: 4 blocks = 512 cols per matmul) ----------
        ones512 = sing.tile([1, 512], BF16, name="ones512")
        nc.vector.memset(ones512[:], 1.0)

        # PE p-state warm-up: sustained matmul activity ahead of the
        # latency-sensitive encoder chain so the PE clock ramps off cold
        for _ in range(24):
            wrm = aggp.tile([128, 4, 128], F32, name="wrm", tag="agg")
            nc.tensor.matmul(wrm[:, 0:4, :], lhsT=ident[:],
                             rhs=iotab[:, 0:4, :], start=True, stop=True)

        def enc_group(b0, nb):
            w = nb * 128
            gsl = slice(b0 * 128, b0 * 128 + w)
            xatg = tp.tile([3, 512], F32, name="xatg", tag="xat")
            nc.sync.dma_start(xatg[:, 0:w], P["xat"].ap()[0:3, gsl])
            attg = tp.tile([2, 512], F32, name="attg", tag="att")
            nc.sync.dma_start(attg[:, 0:w], P["xat"].ap()[3:5, gsl])
            ps = aggp.tile([128, 4, 128], F32, name="ps_enc", tag="agg")
            nc.tensor.matmul(ps[:, 0:nb, :], lhsT=sb["se_W"][:],
                             rhs=xatg[:, 0:w], start=True, stop=False)
            nc.tensor.matmul(ps[:, 0:nb, :], lhsT=sb["eb_se"][:],
                             rhs=ones512[:, 0:w], start=False, stop=True)
            nc.scalar.activation(h[:, b0:b0 + nb, 0:128], ps[:, 0:nb, :],
                                 AF.Sigmoid)
            # p0^T = sigmoid(pe_W1^T @ at + b1^T) computed directly
            pp = aggp.tile([128, 4, 128], F32, name="pp_enc", tag="agg")
            nc.tensor.matmul(pp[:, 0:nb, :], lhsT=sb["pe_W1"][:],
                             rhs=attg[:, 0:w], start=True, stop=False)
            nc.tensor.matmul(pp[:, 0:nb, :], lhsT=sb["eb_p1"][:],
                             rhs=ones512[:, 0:w], start=False, stop=True)
            p0t = tp.tile([128, 512], BF16, name="p0t", tag="tt")
            nc.scalar.activation(p0t[:, 0:w], pp[:, 0:nb, :], AF.Sigmoid)
            # note p0t is [f, n]; pe_W2^T @ p0 needs rhs = p0 [f(k), n] = p0t
            pq = aggp.tile([128, 4, 128], F32, name="pq_enc", tag="agg")
            nc.tensor.matmul(pq[:, 0:nb, :], lhsT=sb["pe_W2"][:],
                             rhs=p0t[:, 0:w], start=True, stop=False)
            nc.tensor.matmul(pq[:, 0:nb, :], lhsT=sb["eb_p2"][:],
                             rhs=ones512[:, 0:w], start=False, stop=True)
            nc.scalar.activation(h[:, b0:b0 + nb, 128:256], pq[:, 0:nb, :],
                                 AF.Sigmoid)

        # ---------- rounds ----------
        def emit_z_group(r, b0):
            """z for round r, blocks [b0, b0+7) (h already holds round-r
            input; transposed layout for r==0)."""
            fused = r < 2
            ncol = 256 if fused else 128
            zst = zp.tile([128, ZB, 256], FP8, name="zst", tag="zst")
            for bb in range(7):
                b = b0 + bb
                if r == 0:
                    sT = h[:, b, 0:128]
                    pT = h[:, b, 128:256]
                else:
                    sTt = tp.tile([128, 128], BF16, name="sT", tag="tt")
                    transpose_to(sTt[:], h[:, b, 0:128])
                    sT = sTt[:]
                    pTt = tp.tile([128, 128], BF16, name="pT", tag="tt")
                    transpose_to(pTt[:], h[:, b, 128:256])
                    pT = pTt[:]
                pz = psp.tile([128, ncol], F32, name="pz", tag="enc")
                if fused:
                    Wa = sb["c1"] if r == 0 else sb["c2"]
                    Wb = sb["p1"] if r == 0 else sb["p2"]
                    nc.tensor.matmul(pz[:, 0:128], lhsT=sT, rhs=Wa[:],
                                     start=True, stop=True)
                    nc.tensor.matmul(pz[:, 128:256], lhsT=pT, rhs=Wb[:],
                                     start=True, stop=True)
                else:
                    nc.tensor.matmul(pz[:], lhsT=sT, rhs=sb["ccW0"][:],
                                     start=True, stop=False)
                    nc.tensor.matmul(pz[:], lhsT=pT, rhs=sb["ccW1"][:],
                                     start=False, stop=True)
                nc.scalar.activation(zst[:, bb, 0:ncol], pz[:], AF.Copy,
                                     scale=sb["dinvc"][:, b:b + 1])
            bb = 0
            while bb < 7:
                s = quarter_of(b0 + bb)
                n = min(7 - bb, QBSTART[s + 1] - (b0 + bb))
                nc.sync.dma_start(
                    zlocs[r][s][:].rearrange("(t p) d -> p t d", p=128)
                    [:, b0 + bb - QBSTART[s]:b0 + bb - QBSTART[s] + n, :],
                    zst[:, bb:bb + n, :])
                bb += n

        AG_AT = {2: 0, 6: 1, 10: 2, 14: 3}  # z-group count -> quarter done

        def emit_ag(r, s):
            nc.gpsimd.collective_compute(
                "AllGather", mybir.AluOpType.bypass,
                replica_groups=[list(range(NCORES))],
                ins=[zlocs[r][s][:]], outs=[zgs[r][s][:]])

        def emit_head(b):
            cT = tp.tile([128, 128], BF16, name="cT", tag="tt")
            transpose_to(cT[:], cmb[:, b, 0:128])
            p1_ = psp.tile([128, 128], F32, name="p1h", tag="enc")
            nc.tensor.matmul(p1_[:], lhsT=sb["dw1"][:], rhs=cT[:],
                             start=True, stop=False)
            nc.tensor.matmul(p1_[:], lhsT=sb["db1"][:], rhs=ones_b[:],
                             start=False, stop=True)
            d1t = tp.tile([128, 128], BF16, name="d1t", tag="p0")
            nc.scalar.activation(d1t[:], p1_[:], AF.Sigmoid)
            p2_ = psp.tile([64, 128], F32, name="p2h", tag="enc")
            nc.tensor.matmul(p2_[:], lhsT=sb["dw2"][:], rhs=d1t[:],
                             start=True, stop=False)
            nc.tensor.matmul(p2_[:], lhsT=sb["db2"][:], rhs=ones_b[:],
                             start=False, stop=True)
            d2t = tp.tile([64, 128], BF16, name="d2t", tag="tt")
            nc.scalar.activation(d2t[:], p2_[:], AF.Sigmoid)
            p3_ = psp.tile([128, 1], F32, name="p3h", tag="enc")
            nc.tensor.matmul(p3_[:], lhsT=d2t[:], rhs=sb["dw3"][:],
                             start=True, stop=False)
            nc.tensor.matmul(p3_[:], lhsT=ones_b[:], rhs=sb["db3"][:],
                             start=False, stop=True)
            nc.scalar.activation(outsb[:, b:b + 1], p3_[:], AF.Sigmoid)

        def do_agg(r, on_chunk):
            fused = r < 2
            ncol = 256 if fused else 128
            bias = {0: "cb1", 1: "cb2", 2: "cb3"}[r]

            def load_islab(qi):
                ct0, ct1 = schedule[qi]["ct0"], schedule[qi]["ct1"]
                t = ip.tile([128, (ct1 - ct0) * 8], I16, name="islab",
                            tag="islab")
                nc.sync.dma_start(t[:], P["idx16"].ap()[:, ct0 * 8:ct1 * 8])
                return t
            nxt = load_islab(0)
            for qi, ch in enumerate(schedule):
                ps_of = {}
                # last (seg id, pair-chunk index) per block, for stop=
                last_of = {}
                for g in ch["segs"]:
                    for (b, k0, cnt, pi0) in g["bgroups"]:
                        last_of[b] = (id(g), b, (cnt - 1) // 2)
                banks = [aggp.tile([128, 512], F32, name="pagg", tag="agg")
                         for _ in range((len(ch["blocks"]) + 1) // 2)]
                for i, b in enumerate(ch["blocks"]):
                    pa = banks[i // 2][:, (i % 2) * ncol:(i % 2 + 1) * ncol]
                    ps_of[b] = pa
                    gsl = slice(b * 128, (b + 1) * 128)
                    nc.tensor.matmul(pa, lhsT=sb["sqdr"][:, gsl],
                                     rhs=sb[bias][:, 0:ncol],
                                     start=True, stop=False)
                    zself = zs.tile([128, 256], FP8, name="zself", tag="zself")
                    sq = quarter_of(b)
                    nc.sync.dma_start(
                        zself[:],
                        zlocs[r][sq][:].rearrange("(t p) d -> p t d", p=128)
                        [:, b - QBSTART[sq], :])
                    stop_now = b not in last_of
                    nc.tensor.matmul(pa, lhsT=ident8[:], rhs=zself[:, 0:ncol],
                                     start=False, stop=stop_now)
                ct0 = ch["ct0"]
                islab = nxt
                if qi + 1 < len(schedule):
                    nxt = load_islab(qi + 1)
                for g in ch["segs"]:
                    nt, t0 = g["nt"], g["t0"]
                    # least-loaded queue (descriptor count proxy) keeps the
                    # four Q7 pairs evenly busy
                    q = min(range(NQ), key=lambda i: gq["load"][i])
                    gq["load"][q] += nt
                    G = gp.tile([128, GCAP, 256], FP8, name="G", tag="G")
                    nc.gpsimd.dma_gather(
                        G[:, 0:nt, :], zgs[r][g["s"]][:],
                        islab[:, (t0 - ct0) * 8:(t0 - ct0 + nt) * 8],
                        nt * 128, nt * 128, 256, single_packet=False,
                        queue_num=q)
                    if not g["bgroups"]:
                        continue
                    p0 = g["bgroups"][0][3]
                    np_ = sum(cnt for (_, _, cnt, _) in g["bgroups"])
                    assert np_ <= 2 * GCAP
                    S = sp.tile([128, 2 * GCAP, 128], FP8, name="S", tag="S")
                    nc.vector.tensor_tensor(
                        out=S[:, 0:np_, :],
                        in0=sb["dstslot"][:, p0:p0 + np_].to_broadcast(
                            [128, np_, 128]),
                        in1=iotab[:, 0:np_, :],
                        op=mybir.AluOpType.is_equal)
                    for (b, k0, cnt, pi0) in g["bgroups"]:
                        j0 = pi0 - p0
                        for j in range(0, cnt, 2):
                            stop = last_of[b] == (id(g), b, j // 2)
                            if j + 1 < cnt:
                                nc.tensor.matmul(
                                    ps_of[b],
                                    lhsT=S[:, j0 + j:j0 + j + 2, :],
                                    rhs=G[:, k0 + j:k0 + j + 2, 0:ncol],
                                    start=False, stop=stop, perf_mode=DR)
                            else:
                                nc.tensor.matmul(
                                    ps_of[b], lhsT=S[:, j0 + j, :],
                                    rhs=G[:, k0 + j, 0:ncol],
                                    start=False, stop=stop)
                for b in ch["blocks"]:
                    dst = h[:, b, 0:256] if fused else cmb[:, b, 0:128]
                    nc.scalar.activation(dst, ps_of[b], AF.Relu,
                                         scale=sb["dinvc"][:, b:b + 1])
                on_chunk(qi)

        zg0 = 0
        for b0 in range(0, NBLK, 4):
            nb = min(4, NBLK - b0)
            enc_group(b0, nb)
            while zg0 < 14 and 7 * zg0 + 7 <= b0 + nb:
                emit_z_group(0, 7 * zg0)
                zg0 += 1
                if zg0 in AG_AT:
                    emit_ag(0, AG_AT[zg0])

        for r in range(3):
            state = dict(zg=0, hd=0)

            def on_chunk(qi, r=r, state=state):
                ready = min(CB * (qi + 1), NBLK)
                if r < 2:
                    while state["zg"] < 14 and 7 * state["zg"] + 7 <= ready:
                        emit_z_group(r + 1, 7 * state["zg"])
                        state["zg"] += 1
                        if state["zg"] in AG_AT:
                            emit_ag(r + 1, AG_AT[state["zg"]])
                else:
                    while state["hd"] < NBLK and state["hd"] < ready:
                        emit_head(state["hd"])
                        state["hd"] += 1

            do_agg(r, on_chunk)

        nc.sync.dma_start(out_p.ap(), outsb[:])
    nc.compile()
    return nc


def kernel(**inputs):
    inputs = {k: np.asarray(v) for k, v in inputs.items()}
    pre = _preprocess(inputs["edge_index"])
    nc = _build(pre["schedule"], pre["TT"], pre["NSEG"], pre["NPAIR"])

    x = inputs["x"].astype(np.float32)
    alpha = inputs["alpha"].astype(np.float32)
    torque = inputs["torque"].astype(np.float32)

    def bf(a):
        return np.ascontiguousarray(np.asarray(a, np.float32).astype(BF))

    np8 = mybir.dt.np(FP8)
    iota_host = np.tile(np.arange(128, dtype=np.float32)[None, None, :],
                        (128, 2 * GCAP, 1)).reshape(128, 2 * GCAP * 128)
    shared = dict(
        se_W=np.ascontiguousarray(inputs["se_W"].astype(np.float32)),
        pe_W1=np.ascontiguousarray(inputs["pe_W1"].astype(np.float32)),
        pe_W2=bf(inputs["pe_W2"]), c1=bf(inputs["c1_W"]), p1=bf(inputs["p1_W"]),
        c2=bf(inputs["c2_W"]), p2=bf(inputs["p2_W"]),
        ccW0=bf(inputs["cc_W"][0:128]), ccW1=bf(inputs["cc_W"][128:256]),
        dw1=bf(inputs["dp_W1"]), dw2=bf(inputs["dp_W2"]), dw3=bf(inputs["dp_W3"]),
        eb_se=bf(inputs["se_b"][None, :]), eb_p1=bf(inputs["pe_b1"][None, :]),
        eb_p2=bf(inputs["pe_b2"][None, :]),
        cb1=bf(np.concatenate([inputs["c1_b"], inputs["p1_b"]])[None, :]),
        cb2=bf(np.concatenate([inputs["c2_b"], inputs["p2_b"]])[None, :]),
        cb3=bf(inputs["cc_b"][None, :]),
        db1=bf(inputs["dp_b1"][None, :]), db2=bf(inputs["dp_b2"][None, :]),
        db3=bf(inputs["dp_b3"][None, :]),
        iotab=bf(iota_host),
        ident=bf(np.eye(128, dtype=np.float32)),
        ident8=np.ascontiguousarray(np.eye(128, dtype=np.float32).astype(np8)),
    )

    in_maps = []
    for c in range(NCORES):
        sl = slice(c * NC, (c + 1) * NC)
        xat = np.zeros((5, PADN), np.float32)
        xat[0:3, :NC] = x[sl].T
        xat[3, :NC] = alpha[sl, 0]
        xat[4, :NC] = torque[sl, 0]
        dinvc = np.ascontiguousarray(pre["dinv_p"][c].reshape(NBLK, 128).T)
        in_maps.append(dict(
            shared,
            xat=xat, dinvc=dinvc,
            sqdr=np.ascontiguousarray(pre["sqd_p"][c][None, :].astype(BF)),
            idx16=np.ascontiguousarray(pre["idx16"][c]),
            dstslot=np.ascontiguousarray(pre["dstslot"][c].astype(BF)),
        ))

    import os
    trace = os.environ.get("KERNEL_PROFILE", "") == "1"
    res = run_bass_kernel_spmd(nc, in_maps, list(range(NCORES)), trace=trace)
    global LAST_EXEC_NS, LAST_RESULTS
    LAST_EXEC_NS = res.exec_time_ns
    LAST_RESULTS = res
    out = np.empty((N, 1), np.float32)
    for c in range(NCORES):
        o = res.results[c]["out"]  # [128, NBLK]
        out[c * NC:(c + 1) * NC, 0] = o.T.reshape(-1)[:NC]
    return out
